# revision 65
# baseline (speedup 1.0000x reference)
"""Multi-head causal attention (B=2, T=4096, H=8, D=64) on 8 TRN2 NeuronCores.

Sharding: core c handles batch b = c//4 and heads (2*(c%4), 2*(c%4)+1); the
host sums the four per-batch partials and adds b_proj.

Design (driven by the TimelineSim cost model, where matmul cost = output
free-dim rows and the exp() on ACT is the binding resource at ~141us):
- QKV: fp16 x and weights (halves input DMA, fp16 matmuls run at bf16
  speed with 8x finer mantissa); q/k accumulate in f32 PSUM and are
  staged to SBUF as f32r for the score matmuls.
- S^T per 128-key chunk: [keys, queries] f32r matmuls into two 3-bank
  PSUM slots; the causal triangle is applied ON THE PE by accumulating
  (-1e5*I) @ I(k>q') onto the diagonal 128x128 block - no cross-engine
  mask pass ever touches the score pipeline.
- exp on ACT with bias=-3 (cancels exactly in softmax; keeps P in
  range) writing P to SBUF; P feeds the PV matmuls as the stationary
  operand: out[q,d] += P[k,q].T @ Vaug[k,65] costs 65 rows per
  128x128 pair instead of 512 per chunk (the ones-column yields the
  softmax denominators for free).
- The four per-q-chunk PV accumulators share one PSUM bank
  ([128,4,65]); the bank is memset once and every PV matmul uses
  start=False, dodging the 2KB PSUM zero-region clobber.
- normalize: per-partition reciprocal + tensor_scalar_mul into
  [q, 2*64] fp16 tiles, transposed to [d, q] via DMA XBAR transposes
  (PE transposes through the freed S slots for the final tile), then
  one [128d -> 512] projection per t-chunk; fp16 partial out.
- Scheduling: engines are strictly in-order and an instruction waiting
  on a semaphore blocks everything behind it, so exp is emitted 1 block
  behind S and PV 1 block behind exp; QKV/projection work is dripped
  between blocks as ready filler; all input DMAs are issued up front; a
  PE warm-up burst covers the p-state ramp during the first DMAs.
- fp8e4+DoubleRow PV (2x cheaper in the cost model) is implemented
  behind ATTN_PV_FP8=1 but crashes the device runtime, so bf16 P/V is
  the default.
"""

import os
import sys

for _p in ("/opt/trn_rl_repo", "/root/.axon_site/_ro/trn_rl_repo"):
    if os.path.isdir(_p) and _p not in sys.path:
        sys.path.insert(0, _p)
        break

from contextlib import ExitStack

import ml_dtypes
import numpy as np

B, T, H, D = 2, 4096, 8, 64
C = H * D  # 512
NQT = T // 512  # 8 q-tiles of 512 queries
NKC = T // 128  # 32 k-chunks of 128 keys

PV_BF16 = os.environ.get("ATTN_PV_FP8") != "1"  # fp8/DR breaks device exec
NO_DMAT = os.environ.get("ATTN_NO_DMAT") == "1"   # fallback: PE transposes only
NEG = -1.0e5
EXP_BIAS = -3.0

_cache = {}


def _build(has_bias=True):
    import concourse.mybir as mybir
    import concourse.tile as tile
    from concourse import bacc

    f32 = mybir.dt.float32
    f32r = mybir.dt.float32r
    bf16 = mybir.dt.bfloat16
    fp16 = mybir.dt.float16
    fp8 = mybir.dt.float8e4
    pdt = bf16 if PV_BF16 else fp8
    DR = None if PV_BF16 else mybir.MatmulPerfMode.DoubleRow
    Exp = mybir.ActivationFunctionType.Exp

    nc = bacc.Bacc("TRN2", target_bir_lowering=False, debug=False,
                   enable_asserts=False)

    xt_d = nc.dram_tensor("xt", [128, 4, T], fp16, kind="ExternalInput").ap()
    wqk_d = nc.dram_tensor("wqk", [128, 4, 256], fp16,
                           kind="ExternalInput").ap()
    wv_d = nc.dram_tensor("wv", [128, 4, 128], fp16,
                          kind="ExternalInput").ap()
    wp_d = nc.dram_tensor("wp", [128, C + 128], fp16, kind="ExternalInput").ap()
    tri_d = nc.dram_tensor("tri", [128, 256], bf16, kind="ExternalInput").ap()
    bqk_d = nc.dram_tensor("bqk", [128, 2], f32, kind="ExternalInput").ap()
    bv_d = nc.dram_tensor("bv", [1, 128], fp16, kind="ExternalInput").ap()
    out_d = nc.dram_tensor("partial", [128, NKC, C], fp16,
                           kind="ExternalOutput").ap()

    with tile.TileContext(nc, trace_sim=False) as tc, ExitStack() as ctx:
        cp = ctx.enter_context(tc.tile_pool(name="const", bufs=1))
        sp = ctx.enter_context(tc.tile_pool(name="spsum", bufs=1,
                                            space="PSUM"))
        ap_pool = ctx.enter_context(tc.tile_pool(name="accp", bufs=1,
                                                 space="PSUM"))
        op = ctx.enter_context(tc.tile_pool(name="opsum", bufs=1,
                                            space="PSUM"))
        pp = ctx.enter_context(tc.tile_pool(name="pbuf", bufs=2))
        wk = ctx.enter_context(tc.tile_pool(name="wrk", bufs=2))

        def const(shape, dt, tag):
            return cp.tile(shape, dt, tag=tag, name=tag)

        xb = const([128, 4, T], fp16, "xb")
        wqk = const([128, 4, 256], fp16, "wqk")
        wv = const([128, 4, 128], fp16, "wv")
        wpf = const([128, C + 128], fp16, "wpf")
        trit = const([128, 256], bf16, "trit")
        bqk = const([128, 2], f32, "bqk")
        bv = const([1, 128], fp16, "bv")
        ones1 = const([1, 128], fp16, "ones1")
        ebias = const([128, 1], f32, "ebias")
        qT = const([128, T], f32r, "qT")
        kT = const([128, T], f32r, "kT")
        wup = const([128, 512], fp16, "wup")
        va = [const([128, NKC, 65], pdt, f"va{h}") for h in range(2)]
        vab = [const([128, 4, 65], bf16, f"vab{h}") for h in range(2)]
        oTS = const([128, T], fp16, "oTS")

        # All input DMAs issued up front (nothing later should queue behind
        # the long-waiting XBAR transposes on the SP sequencer)
        nc.sync.dma_start(wqk[:], wqk_d[:])
        nc.sync.dma_start(xb[:, :, 0:512], xt_d[:, :, 0:512])
        if has_bias:
            nc.sync.dma_start(bqk[:], bqk_d[:])
            nc.sync.dma_start(bv[:], bv_d[:])
        nc.sync.dma_start(wv[:], wv_d[:])
        nc.sync.dma_start(trit[:], tri_d[:])
        nc.sync.dma_start(wpf[:], wp_d[:])
        for t in range(1, NQT):
            sn = slice(t * 512, (t + 1) * 512)
            nc.sync.dma_start(xb[:, :, sn], xt_d[:, :, sn])
        nc.vector.memset(wup[:], 0.0)
        nc.vector.memset(ones1[:], 1.0)
        nc.vector.memset(ebias[:], EXP_BIAS)
        nc.vector.memset(va[0][:, :, 64:65], 1.0)
        nc.vector.memset(va[1][:, :, 64:65], 1.0)
        nc.vector.memset(vab[0][:, :, 64:65], 1.0)
        nc.vector.memset(vab[1][:, :, 64:65], 1.0)

        # PE p-state warm-up while the first DMAs land: back-to-back matmuls
        # on a zeroed tile bring the array toward full clock before the
        # first real matmul issues
        psW = ap_pool.tile([128, 512], f32, tag="acc", name="psW")
        for _ in range(7):
            nc.tensor.matmul(psW[:], lhsT=wup[:, 0:128], rhs=wup[:],
                             start=True, stop=True, skip_group_check=True)

        pending = []

        def emit_qk(g, t):
            ps = ap_pool.tile([128, 512], f32, tag="acc", name=f"qk{g}_{t}")
            for ci in range(4):
                nc.tensor.matmul(
                    ps[:],
                    lhsT=wqk[:, ci, g * 128:(g + 1) * 128],
                    rhs=xb[:, ci, t * 512:(t + 1) * 512],
                    start=(ci == 0), stop=(ci == 3))
            dst = (qT if g == 0 else kT)[:, t * 512:(t + 1) * 512]
            if has_bias:
                nc.vector.tensor_scalar_add(dst, ps[:], bqk[:, g:g + 1])
            elif g == 1 and t == 0:
                # ACT is idle before the first exp; unserialize the two
                # copies the first S-block waits on
                nc.scalar.copy(dst, ps[:])
            else:
                nc.vector.tensor_copy(dst, ps[:])

        def emit_v(tv):
            ps = ap_pool.tile([128, 512], f32, tag="acc", name=f"v{tv}")
            psv = ps[:, 0:128]
            o = tv * 128
            for ci in range(4):
                nc.tensor.matmul(
                    psv, lhsT=xb[:, ci, o:o + 128],
                    rhs=wv[:, ci, :], start=(ci == 0),
                    stop=(not has_bias and ci == 3))
            if has_bias:
                nc.tensor.matmul(psv, lhsT=ones1[:], rhs=bv[:],
                                 start=False, stop=True)
            for h in range(2):
                nc.vector.tensor_copy(va[h][:, tv, 0:64],
                                      psv[:, h * 64:h * 64 + 64])
                if tv < 4:
                    nc.vector.tensor_copy(vab[h][:, tv, 0:64],
                                          psv[:, h * 64:h * 64 + 64])

        ob = {}

        def emit_proj(tc_):
            qi, j = tc_ // 4, tc_ % 4
            last = qi == NQT - 1
            if last:
                # last tile: S slots are free - run the four projections on
                # the two slot rings in parallel, copies alternating DVE/ACT
                psP = sp.tile([128, 1536], f32, tag=f"s{j % 2}",
                              name=f"pj{tc_}")[:, 0:512]
            else:
                psP = ap_pool.tile([128, 512], f32, tag="acc",
                                   name=f"pj{tc_}")
            nc.tensor.matmul(psP, lhsT=oTS[:, tc_ * 128:(tc_ + 1) * 128],
                             rhs=wpf[:, 0:C], start=True, stop=True)
            if last:
                obt = wk.tile([128, 512], fp16, tag="obL", bufs=4,
                              name=f"obL{tc_}")
                if j % 2 == 0:
                    nc.vector.tensor_copy(obt[:], psP)
                else:
                    nc.scalar.copy(obt[:], psP)
                nc.sync.dma_start(out_d[:, tc_, :], obt[:])
            else:
                if j == 0:
                    ob[qi] = wk.tile([128, 4, 512], fp16, tag="ob", bufs=2,
                                     name=f"ob{qi}")
                nc.vector.tensor_copy(ob[qi][:, j, :], psP)
                if j == 3:
                    nc.sync.dma_start(out_d[:, qi * 4:(qi + 1) * 4, :],
                                      ob[qi][:])

        # ---- attention blocks: 2 chunks each, 3 rotating PSUM slots ----
        class Blk:
            pass

        blocks = []
        gslot = 0
        for qi in range(NQT):
            for h in range(2):
                nkc = 4 * qi + 4
                rem = nkc % 3
                lens = ([rem] if rem else []) + [3] * (nkc // 3)
                kc = 0
                for ln in lens:
                    b = Blk()
                    b.h, b.qi, b.kc = h, qi, kc
                    b.len = ln
                    b.slot = gslot
                    b.first = kc == 0
                    b.last = kc + ln == nkc
                    blocks.append(b)
                    gslot = (gslot + 1) % 2
                    kc += ln

        psO = {}
        Ptile = {}
        oN = {}

        def emit_s(b):
            hb = b.h * 64
            b.psS = sp.tile([128, 1536], f32, tag=f"s{b.slot}", bufs=1,
                            name=f"s_{b.h}_{b.qi}_{b.kc}")
            for j in range(b.len):
                kc = b.kc + j
                p = kc - 4 * b.qi
                nc.tensor.matmul(
                    b.psS[:, j * 512:(j + 1) * 512],
                    lhsT=kT[hb:hb + 64, kc * 128:(kc + 1) * 128],
                    rhs=qT[hb:hb + 64,
                           b.qi * 512:(b.qi + 1) * 512],
                    start=True, stop=True)
                if p >= 0:
                    # accumulate -1e5 * I(k > q') onto the causal triangle
                    to = j * 512 + 128 * p
                    nc.tensor.matmul(
                        b.psS[:, to:to + 128],
                        lhsT=trit[:, 0:128], rhs=trit[:, 128:256],
                        start=False, stop=True, skip_group_check=True)

        def emit_exp(b):
            h, qi = b.h, b.qi
            key = (h, qi)
            if b.first:
                if qi == 0:
                    Ptile[key] = pp.tile([128, 4, 512], bf16, tag="P0",
                                         name=f"p0_{h}")
                else:
                    Ptile[key] = pp.tile([128, NKC, 512], pdt, tag="P",
                                         name=f"p_{h}_{qi}")
            P3 = Ptile[key]
            Pflat = P3.rearrange("p a b -> p (a b)")
            p0 = b.kc - 4 * qi
            off0 = 128 * p0 if p0 in (1, 2) else 0
            nc.scalar.activation(
                Pflat[:, b.kc * 512 + off0:(b.kc + b.len) * 512],
                b.psS[:, off0:b.len * 512], Exp, bias=ebias[:])

        def emit_pv(b):
            h, qi = b.h, b.qi
            key = (h, qi)
            if b.first:
                psO[key] = op.tile([128, 4, 65], f32, tag="psO",
                                   name=f"o_{h}_{qi}")
                nc.vector.memset(psO[key][:], 0.0)
            P3 = Ptile[key]
            po = psO[key]
            vah = vab[h] if qi == 0 else va[h]
            stopped = []
            for j in range(4):
                kmax = 4 * qi + j
                if PV_BF16 or qi == 0:
                    for kc in range(b.kc, min(b.kc + b.len, kmax + 1)):
                        nc.tensor.matmul(
                            po[:, j, :],
                            lhsT=P3[:, kc, j * 128:(j + 1) * 128],
                            rhs=vah[:, kc, :],
                            start=False, stop=(kc == kmax),
                            skip_group_check=True)
                        if kc == kmax:
                            stopped.append(j)
                    continue
                # fp8 DoubleRow over chunk pairs; odd leftover as single
                for kc2 in range(b.kc, b.kc + b.len):
                    if kc2 % 2 == 1 and kc2 <= kmax:
                        m = kc2 - 1
                        nc.tensor.matmul(
                            po[:, j, :],
                            lhsT=P3[:, m:m + 2, j * 128:(j + 1) * 128],
                            rhs=vah[:, m:m + 2, :],
                            start=False, stop=(kc2 == kmax),
                            perf_mode=DR, skip_group_check=True)
                        if kc2 == kmax:
                            stopped.append(j)
                    if kc2 == kmax and kmax % 2 == 0:
                        nc.tensor.matmul(
                            po[:, j, :],
                            lhsT=P3[:, kmax, j * 128:(j + 1) * 128],
                            rhs=vah[:, kmax, :],
                            start=False, stop=True,
                            skip_group_check=True)
                        stopped.append(j)

            # per-q-chunk normalize as soon as its accumulation stopped
            for j in stopped:
                rec = wk.tile([128, 1], f32, tag="rec", bufs=4,
                              name=f"rc{h}_{qi}_{j}")
                nc.vector.reciprocal(rec[:], po[:, j, 64:65])
                if h == 0:
                    oN[(qi, j)] = wk.tile([128, 128], fp16, tag=f"oN{j}",
                                          bufs=2, name=f"oN{qi}_{j}")
                nc.vector.tensor_scalar_mul(
                    oN[(qi, j)][:, h * 64:(h + 1) * 64],
                    po[:, j, 0:64], rec[:])
                if h == 1:
                    tc_ = 4 * qi + j
                    if NO_DMAT or (qi == NQT - 1 and j >= 2):
                        # tail: PE transpose + DVE copy beats the DMA
                        # transpose's hwdge+sem latency
                        psT = sp.tile([128, 1536], f32, tag=f"s{j % 2}",
                                      name=f"tr{tc_}")
                        psTb = psT[:, 0:64].bitcast(fp16)
                        nc.tensor.transpose(psTb, oN[(qi, j)][:],
                                            wpf[:, C:C + 128])
                        nc.vector.tensor_copy(
                            oTS[:, tc_ * 128:(tc_ + 1) * 128], psTb)
                        emit_proj(tc_)
                    else:
                        nc.sync.dma_start(
                            oTS[:, tc_ * 128:(tc_ + 1) * 128],
                            oN[(qi, j)][:], transpose=True)
                        if qi == NQT - 1:
                            emit_proj(tc_)
                        else:
                            pending.append(lambda tc_=tc_: emit_proj(tc_))

        def emit_inputs(t):
            if t >= NQT:
                return
            pending.append(lambda: emit_qk(0, t))
            pending.append(lambda: emit_qk(1, t))
            for tv in range(4 * t, 4 * t + 4):
                pending.append(lambda tv=tv: emit_v(tv))

        # prologue: only the QK path for tile 0 inline; V work is dripped
        emit_qk(0, 0)
        emit_qk(1, 0)
        for tv in range(0, 4):
            pending.append(lambda tv=tv: emit_v(tv))

        from collections import deque
        expq = deque()
        pvq = deque()
        LA_EXP = 1   # exp lags S-block emission by this many blocks
        LA_PV = 1    # PV lags its exp by this many more blocks
        bi = 0
        for t in range(NQT):
            if pending:
                for fn in pending:
                    fn()
                pending.clear()
            emit_inputs(t + 1)
            while bi < len(blocks) and blocks[bi].qi == t:
                b = blocks[bi]
                # S first: its matmuls gate the next exp on ACT
                emit_s(b)
                expq.append(b)
                if len(expq) > LA_EXP:
                    eb = expq.popleft()
                    emit_exp(eb)
                    pvq.append(eb)
                if len(pvq) > LA_PV:
                    emit_pv(pvq.popleft())
                npop = 4 if t <= 2 else 1
                for _ in range(min(npop, len(pending))):
                    pending.pop(0)()
                bi += 1
        while expq:
            eb = expq.popleft()
            emit_exp(eb)
            pvq.append(eb)
            if len(pvq) > 1:
                emit_pv(pvq.popleft())
        while pvq:
            emit_pv(pvq.popleft())
        for fn in pending:
            fn()
        pending.clear()

    nc.compile()
    return nc


def _get_nc(has_bias=False):
    key = f"nc{has_bias}"
    if key not in _cache:
        _cache[key] = _build(has_bias)
    return _cache[key]


def _prep_inputs(x, w_qkv, b_qkv, w_proj):
    x = np.asarray(x, np.float32)
    w_qkv = np.asarray(w_qkv, np.float32)
    b_qkv = np.asarray(b_qkv, np.float32)
    bf = ml_dtypes.bfloat16

    # tri tile: [:, 0:128] = -1e5 * identity (stationary), [:, 128:256] =
    # I(k > q') (moving) - their matmul adds -1e5 on the causal triangle
    k_idx = np.arange(128)[:, None]
    q_idx = np.arange(128)[None, :]
    tri = np.zeros((128, 256), np.float32)
    tri[:, 0:128] = NEG * np.eye(128, dtype=np.float32)
    tri[:, 128:256] = (k_idx > q_idx).astype(np.float32)
    tri = tri.astype(bf)

    in_maps = []
    for c in range(8):
        b = c // 4
        h0 = 2 * (c % 4)
        cols = slice(h0 * 64, (h0 + 2) * 64)  # 128 contiguous dims (2 heads)
        # [128, 4, T]: xt[k, ci, t] = x[b, t, ci*128 + k]
        xt = np.ascontiguousarray(
            x[b].T.reshape(4, 128, T).transpose(1, 0, 2).astype(np.float16))
        wq = w_qkv[:, :C][:, cols] * 0.125
        wkk = w_qkv[:, C:2 * C][:, cols]
        wvv = w_qkv[:, 2 * C:][:, cols]
        bq = b_qkv[:C][cols] * 0.125
        bk = b_qkv[C:2 * C][cols]
        bvv = b_qkv[2 * C:][cols]
        wqk = np.concatenate([wq, wkk], axis=1)  # [512, 256]
        in_maps.append({
            "xt": xt,
            "wqk": np.ascontiguousarray(
                wqk.reshape(4, 128, 256).transpose(1, 0, 2).astype(np.float16)),
            "wv": np.ascontiguousarray(
                wvv.reshape(4, 128, 128).transpose(1, 0, 2).astype(np.float16)),
            "wp": np.ascontiguousarray(np.concatenate(
                [np.asarray(w_proj, np.float32)[cols, :],
                 np.eye(128, dtype=np.float32)], axis=1).astype(np.float16)),
            "tri": tri,
            "bqk": np.ascontiguousarray(np.stack([bq, bk], axis=1)),
            "bv": np.ascontiguousarray(bvv[None, :].astype(np.float16)),
        })
    return in_maps


def kernel(x, w_qkv, b_qkv, w_proj, b_proj, _want_trace=False):
    from concourse.bass_utils import run_bass_kernel_spmd

    has_bias = bool(np.any(np.asarray(b_qkv)))
    nc = _get_nc(has_bias)
    in_maps = _prep_inputs(x, w_qkv, b_qkv, w_proj)
    res = run_bass_kernel_spmd(nc, in_maps, list(range(8)),
                               trace=_want_trace)
    if _want_trace:
        _cache["last_result"] = res
    out = np.zeros((B, T, C), np.float32)
    for c in range(8):
        part = np.asarray(res.results[c]["partial"], np.float32)
        # [128, 32, 512] -> [T, C]
        out[c // 4] += part.transpose(1, 0, 2).reshape(T, C)
    out += np.asarray(b_proj, np.float32)[None, None, :]
    return out
